# revision 1
# baseline (speedup 1.0000x reference)
"""Trainium2 Bass kernel for retrieval-KNN attention (nn_MAM_68418829025563).

Math (reference):
    query  = x @ w1.T + b1                       # [B, D]
    key    = keys @ w2.T + b2                    # [B, K, D]
    scores = (query . key) / sqrt(D)             # [B, K]
    attn   = softmax(scores, axis=-1)
    out    = 0.5*x + 0.5 * sum_k attn[:,k] * values[:,k,:]

Algebraic refactor (avoids the B*K*D*D key projection; factor K less compute):
    scores[b,k] = (q2[b] . keys[b,k] + s0[b]) / sqrt(D)
    q2 = x @ W + c   with W = w1.T @ w2, c = b1 @ w2      (host-folded weights)
    s0 = x @ u + c0  with u = w1.T @ b2, c0 = b1 . b2

Device mapping (pure data parallel, batch sharded over 8 cores):
  - PE: x transpose, q2/s0 projection, exp-scores shear to block-diagonal,
        attention-weighted value combine (block-diagonal matmul trick).
  - DVE: per-sample products q2*keys (ACT accumulates the dot via the
         activation accumulator), reciprocal, final merge.
  - ACT: PSUM->SBUF copies, exp (+fused denominator accumulation).
  - Kernel is memory-bound: streams keys+values (128 MB/core) at HBM rate.
"""

import math
import os

import numpy as np

B, K, D = 8192, 32, 512
N_CORES = 8
BS = B // N_CORES        # samples per core
P = 128                  # partition tile (samples per b-tile)
NBT = BS // P            # b-tiles per core
NCH = D // P             # contraction chunks of 128
KC = 16                  # keys per DMA chunk
GC = 8                   # value groups (4 samples each) per DMA chunk
NG = P // 4              # groups of 4 samples per b-tile = 32
INV_SQRT_D = 1.0 / math.sqrt(D)
ALPHA = 0.5

_NC_CACHE = {}
LAST_RESULTS = None


def _build_nc():
    import concourse.bass as bass
    import concourse.tile as tile
    from concourse import bacc, mybir

    f32 = mybir.dt.float32
    nc = bacc.Bacc(
        "TRN2",
        target_bir_lowering=False,
        debug=False,
        enable_asserts=False,
        num_devices=N_CORES,
    )

    xs = nc.dram_tensor("xs", [BS, D], f32, kind="ExternalInput").ap()
    keys = nc.dram_tensor("keys", [BS, K, D], f32, kind="ExternalInput").ap()
    values = nc.dram_tensor("values", [BS, K, D], f32, kind="ExternalInput").ap()
    wt = nc.dram_tensor("wt", [P, NCH, D], f32, kind="ExternalInput").ap()
    cvec = nc.dram_tensor("cvec", [1, D], f32, kind="ExternalInput").ap()
    ut = nc.dram_tensor("ut", [P, NCH], f32, kind="ExternalInput").ap()
    c0s = nc.dram_tensor("c0s", [1, 1], f32, kind="ExternalInput").ap()
    smat = nc.dram_tensor("smat", [K, 4, P], f32, kind="ExternalInput").ap()
    ident = nc.dram_tensor("ident", [P, P], f32, kind="ExternalInput").ap()
    out_d = nc.dram_tensor("out", [BS, D], f32, kind="ExternalOutput").ap()

    # values as [(b k), d] rows, partition-major view: vflat2[p, n, d] is flat
    # row n*128+p, so tile n holds 4 consecutive samples' (k, d) rows.
    vflat2 = values.flatten_outer_dims().rearrange("(n p) d -> p n d", p=P)

    mult = mybir.AluOpType.mult
    add = mybir.AluOpType.add

    with tile.TileContext(nc) as tc:
        with (
            tc.tile_pool(name="singles", bufs=1) as singles,
            tc.tile_pool(name="kpool", bufs=2) as kpool,
            tc.tile_pool(name="vpool", bufs=2) as vpool,
            tc.tile_pool(name="xpool", bufs=2) as xpool,
            tc.tile_pool(name="spool", bufs=2) as spool,
            tc.tile_pool(name="opool", bufs=2) as opool,
            tc.tile_pool(name="ps_xt", bufs=1, space="PSUM") as ps_xt,
            tc.tile_pool(name="ps_q2", bufs=1, space="PSUM") as ps_q2,
            tc.tile_pool(name="ps_s0", bufs=1, space="PSUM") as ps_s0,
            tc.tile_pool(name="ps_et", bufs=1, space="PSUM") as ps_et,
            tc.tile_pool(name="ps_l2", bufs=1, space="PSUM") as ps_l2,
            tc.tile_pool(name="ps_cb", bufs=2, space="PSUM") as ps_cb,
        ):
            # --- constants (loaded once) ---
            wt_sb = singles.tile([P, NCH, D], f32)
            nc.sync.dma_start(out=wt_sb, in_=wt)
            cvec_sb = singles.tile([1, D], f32)
            nc.sync.dma_start(out=cvec_sb, in_=cvec)
            ut_sb = singles.tile([P, NCH], f32)
            nc.sync.dma_start(out=ut_sb, in_=ut)
            c0_sb = singles.tile([1, 1], f32)
            nc.sync.dma_start(out=c0_sb, in_=c0s)
            smat_sb = singles.tile([K, 4, P], f32)
            nc.sync.dma_start(out=smat_sb, in_=smat)
            ident_sb = singles.tile([P, P], f32)
            nc.sync.dma_start(out=ident_sb, in_=ident)
            ones_sb = singles.tile([1, P], f32)
            nc.vector.memset(ones_sb, 1.0)
            # G2: per-group zero-padded [128, 64] lhsT tiles for the combine.
            # Group g = 16*beta + j lives at cols [1024*beta + 64*j, +64);
            # its only nonzero columns are 4*j + s (s = 0..3), so the single
            # memset here stays valid across all b-tiles.
            g2_sb = singles.tile([P, 2 * NG * P // 4], f32)  # [128, 2048]
            nc.vector.memset(g2_sb, 0.0)
            g2_view = g2_sb.rearrange("p (b r) -> p b r", b=2)

            for t in range(NBT):
                b0 = t * P

                # --- load x tile, halve it for the final merge ---
                x_tile = xpool.tile([P, D], f32)
                nc.sync.dma_start(out=x_tile, in_=xs[b0 : b0 + P, :])
                x_half = xpool.tile([P, D], f32)
                nc.scalar.mul(out=x_half, in_=x_tile, mul=ALPHA)

                # --- xT via PE transpose ---
                xt_ps = ps_xt.tile([P, NCH, P], f32)
                for j in range(NCH):
                    nc.tensor.transpose(
                        xt_ps[:, j, :], x_tile[:, j * P : (j + 1) * P], ident_sb
                    )
                xt_sb = spool.tile([P, NCH, P], f32, tag="xt_sb")
                nc.scalar.copy(out=xt_sb, in_=xt_ps)

                # --- q2 = x @ W + c ; s0 = x @ u + c0 (PSUM accumulation) ---
                q2_ps = ps_q2.tile([P, D], f32)
                s0_ps = ps_s0.tile([P, 1], f32)
                for j in range(NCH):
                    nc.tensor.matmul(
                        q2_ps, xt_sb[:, j, :], wt_sb[:, j, :],
                        start=(j == 0), stop=False,
                    )
                for j in range(NCH):
                    nc.tensor.matmul(
                        s0_ps, xt_sb[:, j, :], ut_sb[:, j : j + 1],
                        start=(j == 0), stop=False,
                    )
                nc.tensor.matmul(q2_ps, ones_sb, cvec_sb, start=False, stop=True)
                nc.tensor.matmul(s0_ps, ones_sb, c0_sb, start=False, stop=True)
                # q2 pre-scaled by 1/sqrt(D) during the PSUM->SBUF move.
                q2_sb = spool.tile([P, D], f32, tag="q2_sb")
                nc.scalar.mul(out=q2_sb, in_=q2_ps, mul=INV_SQRT_D)
                s0_sb = spool.tile([P, 1], f32, tag="s0_sb")
                nc.scalar.mul(out=s0_sb, in_=s0_ps, mul=INV_SQRT_D)

                # --- scores[b,k] = (q2 . keys[b,k]) / sqrt(D) ---
                # (tensor_tensor_reduce is rejected by this runtime, so:
                #  DVE does the elementwise product, ACT reduces it via the
                #  activation accumulator — the two pipeline across k.)
                scores_sb = spool.tile([P, K], f32, tag="scores")
                for kc0 in range(0, K, KC):
                    ktile = kpool.tile([P, KC, D], f32)
                    nc.sync.dma_start(
                        out=ktile, in_=keys[b0 : b0 + P, kc0 : kc0 + KC, :]
                    )
                    for kl in range(KC):
                        k = kc0 + kl
                        scratch = spool.tile([P, D], f32, tag="scratch")
                        nc.vector.tensor_mul(scratch, q2_sb, ktile[:, kl, :])
                        nc.scalar.activation(
                            out=scratch, in_=scratch,
                            func=mybir.ActivationFunctionType.Copy,
                            accum_out=scores_sb[:, k : k + 1],
                        )

                # --- softmax pieces: E = exp(scores + s0s), denom = sum_k E ---
                e_sb = spool.tile([P, K], f32, tag="e_sb")
                denom_sb = spool.tile([P, 1], f32, tag="denom")
                nc.scalar.activation(
                    out=e_sb, in_=scores_sb,
                    func=mybir.ActivationFunctionType.Exp,
                    bias=s0_sb,
                    accum_out=denom_sb,
                )
                denom2_sb = spool.tile([P, 1], f32, tag="denom2")
                nc.scalar.mul(out=denom2_sb, in_=denom_sb, mul=1.0 / ALPHA)
                rhalf_sb = spool.tile([P, 1], f32, tag="rhalf")
                nc.vector.reciprocal(out=rhalf_sb, in_=denom2_sb)

                # --- shear E into block-diagonal L2 via PE ---
                # L2[32s+k, 32s+g] = E[4g+s, k]; all other entries written 0.
                et_ps = ps_et.tile([K, P], f32)
                nc.tensor.transpose(et_ps, e_sb, ident_sb)
                et_sb = spool.tile([K, P], f32, tag="et_sb")
                nc.scalar.copy(out=et_sb, in_=et_ps)
                et_view = et_sb.rearrange("k (g s4) -> k s4 g", s4=4)
                l2_ps = ps_l2.tile([P, P], f32)
                for s in range(4):
                    nc.tensor.matmul(
                        l2_ps[:, 32 * s : 32 * (s + 1)],
                        smat_sb[:, s, :],
                        et_view[:, s, :],
                        start=True, stop=True,
                    )
                # Scatter L2's nonzero columns into the pre-zeroed G2 tiles:
                # G2 col 1024*beta + 68*j + s  <-  L2 col 32*s + 16*beta + j.
                l2_view = l2_ps.rearrange("p (s4 b q) -> p q b s4", s4=4, b=2)
                for j in range(NG // 2):
                    nc.scalar.copy(
                        out=g2_view[:, :, 68 * j : 68 * j + 4],
                        in_=l2_view[:, j, :, :],
                    )

                # --- combine = sum_k E * values via block-diag matmuls ---
                # Half-block beta accumulates its 16 groups into rows
                # [64*beta, 64*beta+64) of comb_ps.
                comb_ps = ps_cb.tile([P, D], f32)
                for vc in range(0, NG, GC):
                    vtile = vpool.tile([P, GC, D], f32)
                    nc.sync.dma_start(
                        out=vtile,
                        in_=vflat2[:, NG * t + vc : NG * t + vc + GC, :],
                    )
                    for gi in range(GC):
                        g = vc + gi
                        beta, j = divmod(g, NG // 2)
                        nc.tensor.matmul(
                            comb_ps[64 * beta : 64 * (beta + 1), :],
                            g2_sb[:, 1024 * beta + 64 * j : 1024 * beta + 64 * (j + 1)],
                            vtile[:, gi, :],
                            start=(j == 0), stop=(j == NG // 2 - 1),
                        )

                # --- out = 0.5*x + (0.5/denom)*comb ---
                out_sb = opool.tile([P, D], f32)
                nc.vector.scalar_tensor_tensor(
                    out=out_sb,
                    in0=comb_ps,
                    scalar=rhalf_sb,
                    in1=x_half,
                    op0=mult,
                    op1=add,
                )
                nc.sync.dma_start(out=out_d[b0 : b0 + P, :], in_=out_sb)

    nc.compile()
    return nc


def _get_nc():
    if "nc" not in _NC_CACHE:
        _NC_CACHE["nc"] = _build_nc()
    return _NC_CACHE["nc"]


def _host_consts(w1, b1, w2, b2):
    w1 = np.asarray(w1, np.float32)
    b1 = np.asarray(b1, np.float32)
    w2 = np.asarray(w2, np.float32)
    b2 = np.asarray(b2, np.float32)
    W = w1.T @ w2                       # [D, D]
    c = b1 @ w2                         # [D]
    u = w1.T @ b2                       # [D]
    c0 = float(b1 @ b2)
    wt = np.ascontiguousarray(W.reshape(NCH, P, D).transpose(1, 0, 2))
    cvec = np.ascontiguousarray(c.reshape(1, D))
    ut = np.ascontiguousarray(u.reshape(NCH, P).T)
    c0s = np.full((1, 1), c0, np.float32)
    smat = np.zeros((K, 4, P), np.float32)
    for k in range(K):
        for s in range(4):
            smat[k, s, 32 * s + k] = 1.0
    identm = np.eye(P, dtype=np.float32)
    return wt, cvec, ut, c0s, smat, identm


def kernel(x, keys, values, w1, b1, w2, b2):
    global LAST_RESULTS
    from concourse import bass_utils

    x = np.ascontiguousarray(np.asarray(x, np.float32))
    keys = np.ascontiguousarray(np.asarray(keys, np.float32))
    values = np.ascontiguousarray(np.asarray(values, np.float32))
    wt, cvec, ut, c0s, smat, identm = _host_consts(w1, b1, w2, b2)

    nc = _get_nc()
    in_maps = []
    for ci in range(N_CORES):
        sl = slice(ci * BS, (ci + 1) * BS)
        in_maps.append(
            dict(
                xs=x[sl],
                keys=keys[sl],
                values=values[sl],
                wt=wt,
                cvec=cvec,
                ut=ut,
                c0s=c0s,
                smat=smat,
                ident=identm,
            )
        )
    res = bass_utils.run_bass_kernel_spmd(
        nc, in_maps, core_ids=list(range(N_CORES))
    )
    LAST_RESULTS = res
    return np.concatenate([r["out"] for r in res.results], axis=0)



# revision 3
# speedup vs baseline: 1.0093x; 1.0093x over previous
"""Trainium2 Bass kernel for retrieval-KNN attention (nn_MAM_68418829025563).

Math (reference):
    query  = x @ w1.T + b1                       # [B, D]
    key    = keys @ w2.T + b2                    # [B, K, D]
    scores = (query . key) / sqrt(D)             # [B, K]
    attn   = softmax(scores, axis=-1)
    out    = 0.5*x + 0.5 * sum_k attn[:,k] * values[:,k,:]

Algebraic refactor (avoids the B*K*D*D key projection):
    scores[b,k] = q2[b] . keys[b,k] + s0[b]
    q2 = x @ W + c   with W = (w1.T @ w2)/sqrt(D), c = (b1 @ w2)/sqrt(D)
    s0 = x @ u + c0  with u = (w1.T @ b2)/sqrt(D), c0 = (b1 . b2)/sqrt(D)

fp16 data path (inputs cast on host; tolerance 2e-2 >> fp16 error ~1e-3)
halves HBM traffic, which is the roofline: ~67 MiB/core at the 360 GB/s
DMA roofline => ~197 us floor.

Scheduling: every engine's wait queue is FIFO with head-of-line blocking,
so no engine's instruction stream may contain an op that parks long in
front of ops the DMA pipeline depends on. Hence:
  - q2/s0 for tile t+1 are computed during tile t (PE + ACT before exp(t)).
  - The final merge is ACT (comb*1/denom) -> PE (identity-matmul adds
    0.5x) -> ACT (PSUM->SBUF), keeping the DVE stream park-free.
  - All matmul stationary operands are f32/f32r: fp16 weights emit
    InstLdweights which locks the cost model's PE p-state at LOW.
  - out DMAs issue two tiles late so they never head-of-line block SP.
"""

import math

import numpy as np

B, K, D = 8192, 32, 512
N_CORES = 8
BS = B // N_CORES        # samples per core
P = 128                  # partition tile (samples per b-tile)
NBT = BS // P            # b-tiles per core
NCH = D // P             # contraction chunks of 128
KC = 16                  # keys per DMA chunk
NG = P // 4              # groups of 4 samples per b-tile = 32
ALPHA = 0.5

_NC_CACHE = {}
LAST_RESULTS = None
ROLES = {}


def _tag(inst, role):
    try:
        ROLES[inst.ins.name] = role
    except AttributeError:
        pass
    return inst


def _build_nc():
    import concourse.bass as bass
    import concourse.tile as tile
    from concourse import bacc, mybir
    from concourse.ap import AP as _AP

    f32 = mybir.dt.float32
    f32r = mybir.dt.float32r
    f16 = mybir.dt.float16
    nc = bacc.Bacc(
        "TRN2",
        target_bir_lowering=False,
        debug=False,
        enable_asserts=False,
        num_devices=N_CORES,
    )

    x05 = nc.dram_tensor("x05", [BS, D], f16, kind="ExternalInput").ap()
    xt = nc.dram_tensor("xt", [P, NCH, BS], f16, kind="ExternalInput").ap()
    keys = nc.dram_tensor("keys", [BS, K, D], f16, kind="ExternalInput").ap()
    values = nc.dram_tensor("values", [BS, K, D], f16, kind="ExternalInput").ap()
    wt = nc.dram_tensor("wt", [P, NCH, D], f16, kind="ExternalInput").ap()
    cvec = nc.dram_tensor("cvec", [1, D], f16, kind="ExternalInput").ap()
    ut = nc.dram_tensor("ut", [P, NCH], f16, kind="ExternalInput").ap()
    c0s = nc.dram_tensor("c0s", [1, 1], f16, kind="ExternalInput").ap()
    smat = nc.dram_tensor("smat", [K, 4, P], f32r, kind="ExternalInput").ap()
    identm = nc.dram_tensor("identm", [P, P], f32, kind="ExternalInput").ap()
    ident16d = nc.dram_tensor("ident16", [P, P], f16, kind="ExternalInput").ap()
    out_d = nc.dram_tensor("out", [BS, D], f16, kind="ExternalOutput").ap()

    # values as [(b k), d] rows, partition-major view: vflat2[p, n, d] is flat
    # row n*128+p, so tile n holds 4 consecutive samples' (k, d) rows.
    vflat2 = values.flatten_outer_dims().rearrange("(n p) d -> p n d", p=P)

    mult = mybir.AluOpType.mult
    add = mybir.AluOpType.add
    Copy = mybir.ActivationFunctionType.Copy
    Exp = mybir.ActivationFunctionType.Exp

    with tile.TileContext(nc) as tc:
        with (
            tc.tile_pool(name="singles", bufs=1) as singles,
            tc.tile_pool(name="kpool", bufs=2) as kpool,
            tc.tile_pool(name="vpool", bufs=2) as vpool,
            tc.tile_pool(name="v32pool", bufs=2) as v32pool,
            tc.tile_pool(name="ppool", bufs=2) as ppool,
            tc.tile_pool(name="wspool", bufs=1) as wspool,
            tc.tile_pool(name="vlast", bufs=1) as vlast,
            tc.tile_pool(name="xpool", bufs=2) as xpool,
            tc.tile_pool(name="spool", bufs=2) as spool,
            tc.tile_pool(name="opool", bufs=3) as opool,
            tc.tile_pool(name="ps_q2", bufs=2, space="PSUM") as ps_q2,
            tc.tile_pool(name="ps_s0", bufs=1, space="PSUM") as ps_s0,
            tc.tile_pool(name="ps_e4", bufs=1, space="PSUM") as ps_e4,
            tc.tile_pool(name="ps_cb", bufs=2, space="PSUM") as ps_cb,
            tc.tile_pool(name="ps_o", bufs=2, space="PSUM") as ps_o,
        ):
            # --- constants (loaded once) ---
            wt_sb = singles.tile([P, NCH, D], f16)
            nc.sync.dma_start(out=wt_sb, in_=wt)
            xt_sb = singles.tile([P, NCH, BS], f16)
            nc.sync.dma_start(out=xt_sb, in_=xt)
            cvec_sb = singles.tile([1, D], f16)
            nc.sync.dma_start(out=cvec_sb, in_=cvec)
            ut_sb = singles.tile([P, NCH], f16)
            nc.sync.dma_start(out=ut_sb, in_=ut)
            c0_sb = singles.tile([1, 1], f16)
            nc.sync.dma_start(out=c0_sb, in_=c0s)
            smat_sb = singles.tile([K, 4, P], f32r)
            nc.sync.dma_start(out=smat_sb, in_=smat)
            ident_sb = singles.tile([P, P], f32)
            nc.sync.dma_start(out=ident_sb, in_=identm)
            ident16_sb = singles.tile([P, P], f16)
            nc.sync.dma_start(out=ident16_sb, in_=ident16d)
            ones_sb = singles.tile([1, P], f16)
            nc.vector.memset(ones_sb, 1.0)
            # G2: zero-padded [128, 64] lhsT blocks for the combine, one per
            # 4-sample group g. Block g's only nonzero cols are 4*(g%16)+s,
            # so the single memset here stays valid across all b-tiles.
            g2_sb = singles.tile([P, NG, P], f32r)  # [128, 32, 128]
            nc.vector.memset(g2_sb.bitcast(f32), 0.0)
            # Scatter target: block g = 16*beta+j holds its nonzero
            # cols at block-local 64*beta + 4j + s, i.e. flat col
            # 128g + 64*beta + 4j + s = 2112*beta + 132*j + s.
            g2_flat = g2_sb.rearrange("p n r -> p (n r)")
            _base = [list(d) for d in g2_flat.ap]
            g2_scatter = _AP(
                g2_flat.tensor, g2_flat.offset,
                [_base[0], [2112, 2], [132, 16], [1, 4]],
            )

            def q2_block(t):
                """PE projection + ACT copies for tile t's query."""
                b0 = t * P
                q2_ps = ps_q2.tile([P, D], f32)
                s0_ps = ps_s0.tile([P, 1], f32)
                for j in range(NCH):
                    nc.tensor.matmul(
                        q2_ps, xt_sb[:, j, b0 : b0 + P],
                        wt_sb[:, j, :],
                        start=(j == 0), stop=False,
                    )
                nc.tensor.matmul(q2_ps, ones_sb, cvec_sb,
                                 start=False, stop=True)
                for j in range(NCH):
                    nc.tensor.matmul(
                        s0_ps, xt_sb[:, j, b0 : b0 + P],
                        ut_sb[:, j : j + 1],
                        start=(j == 0), stop=False,
                    )
                nc.tensor.matmul(s0_ps, ones_sb, c0_sb,
                                 start=False, stop=True)
                q2_sb = spool.tile([P, D], f16, tag="q2_sb")
                _tag(nc.scalar.copy(out=q2_sb, in_=q2_ps), f"t{t}:q2_copy")
                s0_sb = spool.tile([P, 1], f32, tag="s0_sb")
                nc.scalar.copy(out=s0_sb, in_=s0_ps)
                return q2_sb, s0_sb

            cur_q2, cur_s0 = q2_block(0)
            pending_outs = []  # (dram_slice, sbuf_tile), issued 2 tiles later

            for t in range(NBT):
                b0 = t * P

                # --- input DMAs (SP queue, program order = transfer order) ---
                x_tile = xpool.tile([P, D], f16)
                _tag(nc.sync.dma_start(out=x_tile, in_=x05[b0 : b0 + P, :]),
                     f"t{t}:dma_x")

                # --- scores[b,k] = q2 . keys[b,k]  (DVE only) ---
                scores_sb = spool.tile([P, K], f32, tag="scores")
                prods = []
                for kc in range(K // KC):
                    ktile = kpool.tile([P, KC, D], f16)
                    _tag(nc.sync.dma_start(
                        out=ktile, in_=keys[b0 : b0 + P, kc * KC : (kc + 1) * KC, :]
                    ), f"t{t}:dma_k{kc}")
                    prod = ppool.tile([P, KC, D], f16)
                    q2b = cur_q2.unsqueeze(1).broadcast_to((P, KC, D))
                    _tag(nc.vector.tensor_mul(prod, q2b, ktile), f"t{t}:prod{kc}")
                    prods.append(prod)
                for kc, prod in enumerate(prods):
                    # 3 fp16 pairwise-add levels (2x mode) + one f32 reduce;
                    # levels 2-3 write into the (then dead) product buffer.
                    ws = wspool.tile([P, KC, 256], f16)
                    nc.vector.tensor_add(
                        ws, prod[:, :, 0:256], prod[:, :, 256:512]
                    )
                    nc.vector.tensor_add(
                        prod[:, :, 0:128], ws[:, :, 0:128], ws[:, :, 128:256]
                    )
                    nc.vector.tensor_add(
                        prod[:, :, 128:192], prod[:, :, 0:64], prod[:, :, 64:128]
                    )
                    _tag(nc.vector.tensor_reduce(
                        scores_sb[:, kc * KC : (kc + 1) * KC],
                        prod[:, :, 128:192],
                        axis=mybir.AxisListType.X,
                        op=add,
                    ), f"t{t}:tree{kc}")

                # out(t-2) DMA: data long since ready, never blocks SP.
                if len(pending_outs) >= 2:
                    po_slice, po_tile = pending_outs.pop(0)
                    _tag(nc.sync.dma_start(out=po_slice, in_=po_tile),
                         f"t{t}:dma_out{t-2}")

                # --- q2/s0 for tile t+1, emitted here so the ACT copies sit
                # ahead of exp(t) in ACT's FIFO and the PE matmuls run in the
                # pre-softmax PE idle window. ---
                exp_q2, exp_s0 = cur_q2, cur_s0
                if t + 1 < NBT:
                    cur_q2, cur_s0 = q2_block(t + 1)

                # --- E = exp(scores + s0), denom = sum_k E ---
                e_sb = spool.tile([P, K], f32, tag="e_sb")
                denom_sb = spool.tile([P, 1], f32, tag="denom")
                _tag(nc.scalar.activation(
                    out=e_sb, in_=scores_sb, func=Exp,
                    bias=exp_s0, accum_out=denom_sb,
                ), f"t{t}:exp")
                rdenom_sb = spool.tile([P, 1], f32, tag="rdenom")
                nc.vector.reciprocal(out=rdenom_sb, in_=denom_sb)
                # --- block-diagonal attention matrix via masked-identity
                # matmuls: et4m[32s+k, b] = E[b, k] * [b mod 4 == s]. ---
                comb_ps = ps_cb.tile([P, D], f32)
                if t == NBT - 1:
                    # last tile: warm the PE p-state before the combine (the
                    # tail chain is fully serial, so combine speed matters).
                    # Warm output goes to comb_ps, which the combine's
                    # start=True matmuls fully overwrite afterwards.
                    warm_rhs = g2_sb[:, 0:4, :].rearrange("p n r -> p (n r)")
                    for w in range(55):
                        _tag(nc.tensor.matmul(
                            comb_ps, g2_sb[:, 0, :], warm_rhs,
                            start=True, stop=True,
                        ), f"t{t}:warm{w}")
                # Block-diagonal attention matrix via PE transpose plus
                # f32r shear matmuls (all PSUM writes at partition offset 0:
                # fp32r matmuls reject offset-32/96 destinations).
                # l2[32s+k, 32s+g] = E[4g+s, k].
                etl2_ps = ps_e4.tile([P, 2 * P], f32)
                nc.tensor.transpose(etl2_ps[:K, :P], e_sb, ident_sb)
                et32r_sb = spool.tile([K, P], f32r, tag="et32r")
                nc.vector.tensor_copy(et32r_sb, etl2_ps[:K, :P])
                et_view = et32r_sb.rearrange("k (g s) -> k s g", s=4)
                for s in range(4):
                    nc.tensor.matmul(
                        etl2_ps[:, P + 32 * s : P + 32 * (s + 1)],
                        smat_sb[:, s, :],
                        et_view[:, s, :],
                        start=True, stop=True,
                    )
                # One DVE op scatters 0.5*l2 into the pre-zeroed G2 blocks:
                # g2 col (beta, 68j+s) <- l2 col (32s + 16*beta + j).
                l2_view = etl2_ps[:, P : 2 * P].rearrange(
                    "p (s b j) -> p b j s", s=4, b=2)
                _tag(nc.vector.tensor_scalar_mul(g2_scatter, l2_view, ALPHA),
                     f"t{t}:et4m_copy")
                vlast_sb = None
                if t == NBT - 1:
                    # Dedicated staging for the first DVE upconvert: no WAR
                    # on the combine matmuls.  The one-element WAW guard
                    # keeps it from overtaking the parked scatter in the
                    # DVE exec queue.
                    vlast_sb = vlast.tile([P, KC // 2, D], f32r)
                    nc.vector.tensor_copy(
                        vlast_sb[:, 0, 0:1], etl2_ps[:, P : P + 1])

                # --- combine = 0.5 * sum_k attn * values via zero-padded
                # block matmuls: half-block beta accumulates its 16 groups
                # into rows [64*beta, 64*beta+64) of comb_ps. ---
                for h in range(4):
                    vc, hh = divmod(h, 2)
                    if hh == 0:
                        vtile = vpool.tile([P, KC, D], f16)
                        _tag(nc.sync.dma_start(
                            out=vtile,
                            in_=vflat2[:, NG * t + vc * KC : NG * t + (vc + 1) * KC, :],
                        ), f"t{t}:dma_v{vc}")
                    # fp16 -> f32r upconvert (ACT cannot produce fp32r-
                    # rounded values, so Pool does 3 halves and DVE one; the
                    # last tile ping-pongs Pool/DVE so the tail runs 2-wide).
                    if t == NBT - 1 and h == 2:
                        v32 = vlast_sb
                        conv_eng = nc.vector
                    else:
                        v32 = v32pool.tile([P, KC // 2, D], f32r)
                        if t == NBT - 1:
                            conv_eng = nc.vector if h >= 2 else nc.gpsimd
                        else:
                            conv_eng = nc.vector if h == 3 else nc.gpsimd
                    _tag(conv_eng.tensor_copy(
                        v32, vtile[:, hh * 8 : hh * 8 + 8, :]),
                        f"t{t}:conv{h}")
                    for gi in range(8):
                        g = h * 8 + gi
                        _tag(nc.tensor.matmul(
                            comb_ps,
                            g2_sb[:, g, :],
                            v32[:, gi, :],
                            start=(g == 0), stop=(g == NG - 1),
                        ), f"t{t}:comb{g}")

                # --- out = 0.5*x + comb/denom: ACT scales comb by 1/denom,
                # PE identity-matmuls add the (host-pre-halved) x, ACT copies
                # the result out.  No DVE op ever waits on the combine. ---
                out_sb = opool.tile([P, D], f16)
                if t == NBT - 1:
                    # Last tile: single DVE merge (nothing behind it in the
                    # DVE FIFO at program end, and it is 1.2us shorter).
                    _tag(nc.vector.scalar_tensor_tensor(
                        out=out_sb, in0=comb_ps, scalar=rdenom_sb, in1=x_tile,
                        op0=mult, op1=add,
                    ), f"t{t}:stt")
                else:
                    tmp_sb = spool.tile([P, D], f16, tag="tmp_merge")
                    _tag(nc.scalar.activation(
                        out=tmp_sb, in_=comb_ps, func=Copy, scale=rdenom_sb,
                    ), f"t{t}:scale")
                    out_ps = ps_o.tile([P, D], f32)
                    nc.tensor.matmul(out_ps, ident16_sb, x_tile,
                                     start=True, stop=False)
                    _tag(nc.tensor.matmul(out_ps, ident16_sb, tmp_sb,
                                          start=False, stop=True), f"t{t}:xadd")
                    _tag(nc.scalar.copy(out=out_sb, in_=out_ps), f"t{t}:outcopy")
                pending_outs.append((out_d[b0 : b0 + P, :], out_sb))

            for i, (po_slice, po_tile) in enumerate(pending_outs):
                _tag(nc.sync.dma_start(out=po_slice, in_=po_tile),
                     f"tail:dma_out{NBT - len(pending_outs) + i}")

    nc.compile()
    return nc


def _get_nc():
    if "nc" not in _NC_CACHE:
        _NC_CACHE["nc"] = _build_nc()
    return _NC_CACHE["nc"]


def _host_consts(w1, b1, w2, b2):
    w1 = np.asarray(w1, np.float32)
    b1 = np.asarray(b1, np.float32)
    w2 = np.asarray(w2, np.float32)
    b2 = np.asarray(b2, np.float32)
    inv_sqrt_d = 1.0 / math.sqrt(D)
    W = (w1.T @ w2) * inv_sqrt_d            # [D, D]
    c = (b1 @ w2) * inv_sqrt_d              # [D]
    u = (w1.T @ b2) * inv_sqrt_d            # [D]
    c0 = float(b1 @ b2) * inv_sqrt_d
    wt = np.ascontiguousarray(
        W.reshape(NCH, P, D).transpose(1, 0, 2)).astype(np.float16)
    cvec = np.ascontiguousarray(c.reshape(1, D)).astype(np.float16)
    ut = np.ascontiguousarray(u.reshape(NCH, P).T).astype(np.float16)
    c0s = np.full((1, 1), c0, np.float16)
    # smat[k, s, 32s+k] = 1: shear selector (k -> row 32s+k).
    smat = np.zeros((K, 4, P), np.float32)
    for k in range(K):
        for s in range(4):
            smat[k, s, 32 * s + k] = 1.0
    identm = np.eye(P, dtype=np.float32)
    ident16 = np.eye(P, dtype=np.float16)
    return wt, cvec, ut, c0s, smat, identm, ident16


def make_in_maps(x, keys, values, w1, b1, w2, b2):
    x = np.asarray(x, np.float32)
    keys16 = np.asarray(keys, np.float32).astype(np.float16)
    values16 = np.asarray(values, np.float32).astype(np.float16)
    x05 = (0.5 * x).astype(np.float16)
    wt, cvec, ut, c0s, smat, identm, ident16 = _host_consts(w1, b1, w2, b2)

    in_maps = []
    for ci in range(N_CORES):
        sl = slice(ci * BS, (ci + 1) * BS)
        xc = x[sl]
        # xt[p, j, b] = x[sl][b, j*128+p]
        xt = np.ascontiguousarray(
            xc.T.reshape(NCH, P, BS).transpose(1, 0, 2)).astype(np.float16)
        in_maps.append(
            dict(
                x05=np.ascontiguousarray(x05[sl]),
                xt=xt,
                keys=np.ascontiguousarray(keys16[sl]),
                values=np.ascontiguousarray(values16[sl]),
                wt=wt,
                cvec=cvec,
                ut=ut,
                c0s=c0s,
                smat=smat,
                identm=identm,
                ident16=ident16,
            )
        )
    return in_maps


def postprocess_core_out(arr):
    return np.asarray(arr, np.float16).astype(np.float32)


def kernel(x, keys, values, w1, b1, w2, b2):
    global LAST_RESULTS
    from concourse import bass_utils

    in_maps = make_in_maps(x, keys, values, w1, b1, w2, b2)
    nc = _get_nc()
    res = bass_utils.run_bass_kernel_spmd(
        nc, in_maps, core_ids=list(range(N_CORES))
    )
    LAST_RESULTS = res
    return np.concatenate(
        [postprocess_core_out(r["out"]) for r in res.results], axis=0
    )
